# revision 1
# baseline (speedup 1.0000x reference)
"""Bass/Trainium2 kernel for nn_MOEFeedForward (8-expert top-2 MoE + shared expert).

Sharding: expert-parallel across 8 NeuronCores. Core c holds expert c's
(w1,w2,w3), the full x and gate_w (gate replicated, fp32), and a 1/8 HID-slice
of the shared expert. Each core computes cw[:,c] * FFN_c(x) + shared_slice(x);
the host sums the 8 partial outputs.

Self-contained: hardcodes shapes from the problem spec.
"""
import sys

sys.path.insert(0, "/opt/trn_rl_repo")

from contextlib import ExitStack

import numpy as np
from ml_dtypes import bfloat16

import concourse.bass as bass
import concourse.tile as tile
from concourse import mybir
from concourse.bass_utils import run_bass_kernel_spmd
from concourse.masks import make_identity
from concourse.vector_clock import ScopedClock

DIM = 768
HID = 2048
E = 8
T = 2048
N_CORES = 8
SH = HID // N_CORES  # shared-expert HID slice per core
DC = DIM // 128      # 6 d-chunks
HC = HID // 128      # 16 hid-chunks
SC = SH // 128       # 2 shared hid-chunks
TT = T // 128        # 16 token tiles
TG = T // 512        # 4 token groups (moving dim 512)

F32 = mybir.dt.float32
BF16 = mybir.dt.bfloat16

AF = mybir.ActivationFunctionType
OP = mybir.AluOpType


# ---------------------------------------------------------------------------
# Walrus in this container rejects CTRL instructions (NoOp/Drain) carrying
# more than one sem wait. TileContext's tail drain carries one wait per
# outstanding semaphore. Replace it with a chain of SP nops (one wait each)
# followed by a bare drain.
def _patched_drain_and_barrier(self, tick_clock, wait_clock):
    import bass_rust

    nop_inst = self.nc.sync.nop(nofuse=True, hint="pre_drain_wait_funnel")
    wait_clock.add_sem_waits(
        nop_inst.ins, ScopedClock({None: tick_clock.global_clock})
    )
    si = nop_inst.ins.sync_info
    waits = list(si.on_wait) if si else []
    if len(waits) > 1:
        nop_inst.ins.sync_info.on_wait = waits[:1]
        for w in waits[1:]:
            extra = self.nc.sync.nop(nofuse=True, hint="pre_drain_wait_funnel")
            extra.ins.sync_info = bass_rust.SyncInfo(on_wait=[w], on_update=[])
    self.nc.sync.drain()

    self.nc.all_engine_barrier()
    assert self.sems is not None
    popped = self.nc._tile_sem_poison_stack.pop()
    assert popped is self._sem_poison
    self.nc.clear_and_free_semaphores(list(self.sems.allocated().values()))
    self.nc.all_engine_barrier()


tile.TileContext._drain_and_barrier = _patched_drain_and_barrier


def _split_multi_waits(nc, max_waits=1):
    """This walrus build allows at most one sem wait per instruction. Hoist
    extra waits onto same-engine nops inserted immediately before."""
    import bass_rust

    n_split = 0
    for f in nc.m.functions:
        for bb in f.blocks:
            il = bb.instructions
            i = 0
            while i < len(il):
                inst = il[i]
                si = inst.sync_info
                if si is None or len(si.on_wait) <= max_waits:
                    i += 1
                    continue
                waits = list(si.on_wait)
                si.on_wait = waits[:max_waits]
                for k, w in enumerate(waits[max_waits:]):
                    nop = mybir.InstNoOp(
                        name=f"{inst.name}-wsplit{k}", ins=[], outs=[]
                    )
                    nop.engine = inst.engine
                    nop.sync_info = bass_rust.SyncInfo(on_wait=[w], on_update=[])
                    il.insert(i, nop)
                    i += 1
                n_split += 1
                i += 1
    return n_split
# ---------------------------------------------------------------------------


import os

CFG = {
    "tp_ps": int(os.environ.get("K_TP_PS", "2")),
    "h_ps": int(os.environ.get("K_H_PS", "3")),
    "y_ps": int(os.environ.get("K_Y_PS", "2")),
    "silu": int(os.environ.get("K_SILU", "3")),
    "xtf": int(os.environ.get("K_XTF", "3")),
    "stage": int(os.environ.get("K_STAGE", "2")),
    "yo": int(os.environ.get("K_YO", "3")),
}


def _build_kernel():
    nc = bass.Bass()
    x_d = nc.dram_tensor("x", [T, DIM], F32, kind="ExternalInput")
    gw_d = nc.dram_tensor("gw", [E, DIM], F32, kind="ExternalInput")
    w1_d = nc.dram_tensor("w1", [HID, DIM], BF16, kind="ExternalInput")
    w2_d = nc.dram_tensor("w2", [DIM, HID], BF16, kind="ExternalInput")
    w3_d = nc.dram_tensor("w3", [HID, DIM], BF16, kind="ExternalInput")
    s1_d = nc.dram_tensor("s1", [SH, DIM], BF16, kind="ExternalInput")
    s2_d = nc.dram_tensor("s2", [DIM, SH], BF16, kind="ExternalInput")
    s3_d = nc.dram_tensor("s3", [SH, DIM], BF16, kind="ExternalInput")
    y_d = nc.dram_tensor("y", [T, DIM], F32, kind="ExternalOutput")

    with tile.TileContext(nc) as tc, ExitStack() as ctx:
        const_p = ctx.enter_context(tc.tile_pool(name="const", bufs=1))
        persist = ctx.enter_context(tc.tile_pool(name="persist", bufs=1))
        stage_p = ctx.enter_context(tc.tile_pool(name="stage", bufs=CFG["stage"]))
        silu_p = ctx.enter_context(tc.tile_pool(name="silu", bufs=CFG["silu"]))
        xtf_p = ctx.enter_context(tc.tile_pool(name="xtf", bufs=CFG["xtf"]))
        gate_sb = ctx.enter_context(tc.tile_pool(name="gate_sb", bufs=2))
        h_sb = ctx.enter_context(tc.tile_pool(name="h_sb", bufs=2))
        yo_p = ctx.enter_context(tc.tile_pool(name="yo", bufs=CFG["yo"]))
        tp_ps = ctx.enter_context(tc.tile_pool(name="tp_ps", bufs=CFG["tp_ps"], space="PSUM"))
        gt_ps = ctx.enter_context(tc.tile_pool(name="gt_ps", bufs=1, space="PSUM"))
        h_ps = ctx.enter_context(tc.tile_pool(name="h_ps", bufs=CFG["h_ps"], space="PSUM"))
        y_ps = ctx.enter_context(tc.tile_pool(name="y_ps", bufs=CFG["y_ps"], space="PSUM"))

        ident = const_p.tile([128, 128], F32)
        make_identity(nc, ident)

        # Persistent SBUF tensors (bf16 transposed operands)
        xT = persist.tile([128, DC, T], BF16, tag="xT")           # [d, t]
        w1T = persist.tile([128, DC, HID], BF16, tag="w1T")       # [d, hid]
        w3T = persist.tile([128, DC, HID], BF16, tag="w3T")
        w2T = persist.tile([128, HC, DIM], BF16, tag="w2T")       # [hid, d]
        s1T = persist.tile([128, DC, SH], BF16, tag="s1T")
        s3T = persist.tile([128, DC, SH], BF16, tag="s3T")
        s2T = persist.tile([128, SC, DIM], BF16, tag="s2T")
        gwT = persist.tile([128, DC, E], F32, tag="gwT")          # [d, e]
        cwe = persist.tile([128, TT], F32, tag="cwe")             # per-token gate weight

        # --- gate_w transpose: [8, 768] -> gwT [d, 6, 8] fp32
        gw_sb = const_p.tile([E, DIM], F32, tag="gw_sb")
        nc.sync.dma_start(gw_sb[:], gw_d[:])
        for dc in range(DC):
            pt = tp_ps.tile([128, E], F32, tag="tp")
            nc.tensor.transpose(pt[:], gw_sb[:, dc * 128:(dc + 1) * 128], ident[0:E, 0:E])
            nc.vector.tensor_copy(gwT[:, dc, :], pt[:])

        # --- x: load, transpose (fp32 for gate), cast to bf16 for FFN; gate.
        for ti in range(TT):
            xt_sb = stage_p.tile([128, DIM], F32, tag="xstage")
            nc.sync.dma_start(xt_sb[:], x_d[ti * 128:(ti + 1) * 128, :])
            lg = gt_ps.tile([128, E], F32, tag="gate")
            for dc in range(DC):
                pt = tp_ps.tile([128, 128], F32, tag="tp")
                nc.tensor.transpose(pt[:], xt_sb[:, dc * 128:(dc + 1) * 128], ident[:])
                xf = xtf_p.tile([128, 128], F32, tag="xtf")
                nc.vector.tensor_copy(xf[:], pt[:])
                nc.scalar.copy(xT[:, dc, ti * 128:(ti + 1) * 128], pt[:])
                nc.tensor.matmul(
                    lg[:], xf[:], gwT[:, dc, :],
                    start=(dc == 0), stop=(dc == DC - 1),
                )
            # softmax top-2 -> cwe[:, ti]; expert of this core is gate row 0.
            mx = gate_sb.tile([128, 1], F32, tag="mx")
            nc.vector.reduce_max(out=mx[:], in_=lg[:], axis=mybir.AxisListType.X)
            nmx = gate_sb.tile([128, 1], F32, tag="nmx")
            nc.scalar.mul(nmx[:], mx[:], -1.0)
            ex = gate_sb.tile([128, E], F32, tag="ex")
            nc.scalar.activation(ex[:], lg[:], AF.Exp, bias=nmx[:], scale=1.0)
            # p1 == 1.0 exactly (max logit). Zero it out to find p2.
            m1 = gate_sb.tile([128, E], F32, tag="m1")
            nc.vector.tensor_scalar(m1[:], ex[:], 1.0, None, op0=OP.is_ge)
            ex2 = gate_sb.tile([128, E], F32, tag="ex2")
            nc.vector.tensor_tensor(ex2[:], ex[:], m1[:], op=OP.subtract)
            p2 = gate_sb.tile([128, 1], F32, tag="p2")
            nc.vector.reduce_max(out=p2[:], in_=ex2[:], axis=mybir.AxisListType.X)
            den = gate_sb.tile([128, 1], F32, tag="den")
            nc.vector.tensor_scalar(den[:], p2[:], 1.0, None, op0=OP.add)
            rec = gate_sb.tile([128, 1], F32, tag="rec")
            nc.vector.reciprocal(rec[:], den[:])
            # own expert prob = ex[:, 0]
            e1 = gate_sb.tile([128, 1], F32, tag="e1")
            nc.vector.tensor_scalar(e1[:], ex[:, 0:1], 1.0, None, op0=OP.is_ge)
            e2m = gate_sb.tile([128, 1], F32, tag="e2m")
            nc.vector.tensor_tensor(e2m[:], ex[:, 0:1], p2[:], op=OP.is_equal)
            e2v = gate_sb.tile([128, 1], F32, tag="e2v")
            nc.vector.tensor_tensor(e2v[:], e2m[:], p2[:], op=OP.mult)
            cw_n = gate_sb.tile([128, 1], F32, tag="cw_n")
            nc.vector.tensor_tensor(cw_n[:], e1[:], e2v[:], op=OP.add)
            nc.vector.tensor_tensor(cwe[:, ti:ti + 1], cw_n[:], rec[:], op=OP.mult)

        # --- weight loads via HWDGE DMA-transpose (bf16 DRAM -> transposed SBUF)
        def load_T(src_d, dst, nchunks):
            # dst[:, c, :] = src_d[:, c*128:(c+1)*128].T
            for c in range(nchunks):
                nc.sync.dma_start(
                    dst[:, c, :], src_d[:, c * 128:(c + 1) * 128], transpose=True
                )

        load_T(w1_d, w1T, DC)
        load_T(w3_d, w3T, DC)
        load_T(s1_d, s1T, DC)
        load_T(s3_d, s3T, DC)
        load_T(w2_d, w2T, HC)
        load_T(s2_d, s2T, SC)

        # --- main FFN: per 512-token group
        for tg in range(TG):
            tsl = slice(tg * 512, (tg + 1) * 512)
            hT = h_sb.tile([128, HC, 512], BF16, tag="hT")
            shT = h_sb.tile([128, SC, 512], BF16, tag="shT")
            for (nchunks, a1T, a3T, houtT) in (
                (HC, w1T, w3T, hT),
                (SC, s1T, s3T, shT),
            ):
                for hc in range(nchunks):
                    p1 = h_ps.tile([128, 512], F32, tag="hps")
                    for dc in range(DC):
                        nc.tensor.matmul(
                            p1[:], a1T[:, dc, hc * 128:(hc + 1) * 128], xT[:, dc, tsl],
                            start=(dc == 0), stop=(dc == DC - 1),
                        )
                    p3 = h_ps.tile([128, 512], F32, tag="hps")
                    for dc in range(DC):
                        nc.tensor.matmul(
                            p3[:], a3T[:, dc, hc * 128:(hc + 1) * 128], xT[:, dc, tsl],
                            start=(dc == 0), stop=(dc == DC - 1),
                        )
                    sl = silu_p.tile([128, 512], BF16, tag="silu")
                    nc.scalar.activation(sl[:], p1[:], AF.Silu)
                    nc.vector.tensor_tensor(houtT[:, hc, :], sl[:], p3[:], op=OP.mult)

            # mm3: y[t,d] per 128-token block, d in 2 halves of 384
            for tb in range(4):
                ti = tg * 4 + tb
                tbs = slice(tb * 128, (tb + 1) * 128)
                yo = yo_p.tile([128, DIM], F32, tag="yo")
                for dh in range(2):
                    dsl = slice(dh * 384, (dh + 1) * 384)
                    pe = y_ps.tile([128, 384], F32, tag="y")
                    for hc in range(HC):
                        nc.tensor.matmul(
                            pe[:], hT[:, hc, tbs], w2T[:, hc, dsl],
                            start=(hc == 0), stop=(hc == HC - 1),
                        )
                    ps = y_ps.tile([128, 384], F32, tag="y")
                    for sc in range(SC):
                        nc.tensor.matmul(
                            ps[:], shT[:, sc, tbs], s2T[:, sc, dsl],
                            start=(sc == 0), stop=(sc == SC - 1),
                        )
                    # y = cwe * ye + ys
                    sc_t = yo_p.tile([128, 384], F32, tag="scl")
                    nc.vector.tensor_scalar(sc_t[:], pe[:], cwe[:, ti:ti + 1], None, op0=OP.mult)
                    nc.vector.tensor_tensor(yo[:, dsl], sc_t[:], ps[:], op=OP.add)
                nc.sync.dma_start(y_d[ti * 128:(ti + 1) * 128, :], yo[:])

    _split_multi_waits(nc)
    try:
        _CACHE["makespan_ns"] = max(e[2] for e in tc._perfetto_entries)
    except Exception:
        _CACHE["makespan_ns"] = None
    return nc


_CACHE = {}


def kernel(x, gate_w, w1, w2, w3, ws1, ws2, ws3):
    x = np.asarray(x, dtype=np.float32)
    gate_w = np.ascontiguousarray(np.asarray(gate_w, dtype=np.float32))
    w1 = np.asarray(w1, dtype=np.float32)
    w2 = np.asarray(w2, dtype=np.float32)
    w3 = np.asarray(w3, dtype=np.float32)
    ws1 = np.ascontiguousarray(np.asarray(ws1, dtype=np.float32))
    ws2 = np.asarray(ws2, dtype=np.float32)
    ws3 = np.ascontiguousarray(np.asarray(ws3, dtype=np.float32))

    B, S, D = x.shape
    x2 = np.ascontiguousarray(x.reshape(-1, D))

    if "nc" not in _CACHE:
        _CACHE["nc"] = _build_kernel()
    nc = _CACHE["nc"]

    in_maps = []
    for c in range(N_CORES):
        perm = np.r_[c, np.delete(np.arange(E), c)]
        sh = slice(c * SH, (c + 1) * SH)
        in_maps.append({
            "x": x2,
            "gw": np.ascontiguousarray(gate_w[perm]),
            "w1": np.ascontiguousarray(w1[c].astype(bfloat16)),
            "w2": np.ascontiguousarray(w2[c].astype(bfloat16)),
            "w3": np.ascontiguousarray(w3[c].astype(bfloat16)),
            "s1": np.ascontiguousarray(ws1[sh].astype(bfloat16)),
            "s2": np.ascontiguousarray(ws2[:, sh].astype(bfloat16)),
            "s3": np.ascontiguousarray(ws3[sh].astype(bfloat16)),
        })

    _CACHE["last_in_maps"] = in_maps
    res = run_bass_kernel_spmd(nc, in_maps, list(range(N_CORES)))
    y = np.zeros((T, DIM), dtype=np.float32)
    for c in range(N_CORES):
        y += np.asarray(res.results[c]["y"], dtype=np.float32)
    return y.reshape(B, S, D)



# revision 2
# speedup vs baseline: 2.6540x; 2.6540x over previous
"""Bass/Trainium2 kernel for nn_MOEFeedForward (8-expert top-2 MoE + shared expert).

Sharding: expert-parallel with host-side dispatch. The host computes the gate
(softmax + top-2) and routes tokens: core c receives expert c's tokens (padded
to capacity A) plus a 1/8 token-slice of the shared-expert work (B=256 tokens).
Every core therefore runs ~A+B token-FFNs of identical shape
(hid=2048, dim=768) — balanced, with no 8x dense overcompute. The host applies
the gate weights and scatter-adds the per-core outputs into the full result.

All operands are pre-transposed/laid out on the host so the device does only
contiguous DMAs and back-to-back bf16 matmuls.

Self-contained: hardcodes shapes from the problem spec.
"""
import sys

sys.path.insert(0, "/opt/trn_rl_repo")

from contextlib import ExitStack

import numpy as np
from ml_dtypes import bfloat16

import concourse.bass as bass
import concourse.tile as tile
from concourse import mybir
from concourse.bass_utils import run_bass_kernel_spmd
from concourse.vector_clock import ScopedClock

DIM = 768
HID = 2048
E = 8
T = 2048
N_CORES = 8
B_SH = T // N_CORES  # shared-expert tokens per core (256)
DC = DIM // 128      # 6 d-chunks
HC = HID // 128      # 16 hid-chunks

F32 = mybir.dt.float32
BF16 = mybir.dt.bfloat16

AF = mybir.ActivationFunctionType
OP = mybir.AluOpType


# ---------------------------------------------------------------------------
# Walrus in this container rejects CTRL instructions (NoOp/Drain) carrying
# more than one sem wait. TileContext's tail drain carries one wait per
# outstanding semaphore. Replace it with a chain of SP nops (one wait each)
# followed by a bare drain.
def _patched_drain_and_barrier(self, tick_clock, wait_clock):
    import bass_rust

    nop_inst = self.nc.sync.nop(nofuse=True, hint="pre_drain_wait_funnel")
    wait_clock.add_sem_waits(
        nop_inst.ins, ScopedClock({None: tick_clock.global_clock})
    )
    si = nop_inst.ins.sync_info
    waits = list(si.on_wait) if si else []
    if len(waits) > 1:
        nop_inst.ins.sync_info.on_wait = waits[:1]
        for w in waits[1:]:
            extra = self.nc.sync.nop(nofuse=True, hint="pre_drain_wait_funnel")
            extra.ins.sync_info = bass_rust.SyncInfo(on_wait=[w], on_update=[])
    self.nc.sync.drain()

    self.nc.all_engine_barrier()
    assert self.sems is not None
    popped = self.nc._tile_sem_poison_stack.pop()
    assert popped is self._sem_poison
    self.nc.clear_and_free_semaphores(list(self.sems.allocated().values()))
    self.nc.all_engine_barrier()


tile.TileContext._drain_and_barrier = _patched_drain_and_barrier


def _split_multi_waits(nc, max_waits=1):
    """This walrus build allows at most one sem wait per instruction. Hoist
    extra waits onto same-engine nops inserted immediately before."""
    import bass_rust

    n_split = 0
    for f in nc.m.functions:
        for bb in f.blocks:
            il = bb.instructions
            i = 0
            while i < len(il):
                inst = il[i]
                si = inst.sync_info
                if si is None or len(si.on_wait) <= max_waits:
                    i += 1
                    continue
                waits = list(si.on_wait)
                si.on_wait = waits[:max_waits]
                for k, w in enumerate(waits[max_waits:]):
                    nop = mybir.InstNoOp(
                        name=f"{inst.name}-wsplit{k}", ins=[], outs=[]
                    )
                    nop.engine = inst.engine
                    nop.sync_info = bass_rust.SyncInfo(on_wait=[w], on_update=[])
                    il.insert(i, nop)
                    i += 1
                n_split += 1
                i += 1
    return n_split
# ---------------------------------------------------------------------------


def _build_kernel(A):
    """A: expert-token capacity (multiple of 128). Columns [0, A) use the
    expert weight set; columns [A, A+B_SH) use the shared weight set."""
    N = A + B_SH
    nc = bass.Bass()
    xT_d = nc.dram_tensor("xT", [128, DC, N], BF16, kind="ExternalInput")
    w1_d = nc.dram_tensor("w1T", [128, HC, DC, 128], BF16, kind="ExternalInput")
    w3_d = nc.dram_tensor("w3T", [128, HC, DC, 128], BF16, kind="ExternalInput")
    w2_d = nc.dram_tensor("w2T", [128, HC, DIM], BF16, kind="ExternalInput")
    s1_d = nc.dram_tensor("s1T", [128, HC, DC, 128], BF16, kind="ExternalInput")
    s3_d = nc.dram_tensor("s3T", [128, HC, DC, 128], BF16, kind="ExternalInput")
    s2_d = nc.dram_tensor("s2T", [128, HC, DIM], BF16, kind="ExternalInput")
    y_d = nc.dram_tensor("y", [N, DIM], F32, kind="ExternalOutput")

    # column pieces: (start, len, weight-set) — expert pieces then shared
    pieces = []
    for c0 in range(0, A, 512):
        pieces.append((c0, min(512, A - c0), 0))
    pieces.append((A, B_SH, 1))

    with tile.TileContext(nc) as tc, ExitStack() as ctx:
        persist = ctx.enter_context(tc.tile_pool(name="persist", bufs=1))
        silu_p = ctx.enter_context(tc.tile_pool(name="silu", bufs=3))
        yo_p = ctx.enter_context(tc.tile_pool(name="yo", bufs=2))
        h_ps = ctx.enter_context(tc.tile_pool(name="h_ps", bufs=4, space="PSUM"))
        y_ps = ctx.enter_context(tc.tile_pool(name="y_ps", bufs=3, space="PSUM"))

        xT = persist.tile([128, DC, N], BF16, tag="xT")
        w1T = persist.tile([128, HC, DC, 128], BF16, tag="w1T")
        w3T = persist.tile([128, HC, DC, 128], BF16, tag="w3T")
        s1T = persist.tile([128, HC, DC, 128], BF16, tag="s1T")
        s3T = persist.tile([128, HC, DC, 128], BF16, tag="s3T")
        w2T = persist.tile([128, HC, DIM], BF16, tag="w2T")
        s2T = persist.tile([128, HC, DIM], BF16, tag="s2T")
        # hT holds one piece's activations [hid, piece_cols]
        hT = persist.tile([128, HC, 512], BF16, tag="hT")

        # --- DMA schedule: x first, then expert w1/w3 interleaved by
        # hid-chunk (PE consumes chunk-by-chunk), then w2, then shared.
        nc.sync.dma_start(xT[:], xT_d[:])
        for hc in range(HC):
            nc.sync.dma_start(w1T[:, hc], w1_d[:, hc])
            nc.sync.dma_start(w3T[:, hc], w3_d[:, hc])
        nc.sync.dma_start(w2T[:], w2_d[:])
        nc.sync.dma_start(s1T[:], s1_d[:])
        nc.sync.dma_start(s3T[:], s3_d[:])
        nc.sync.dma_start(s2T[:], s2_d[:])

        for (c0, clen, ws) in pieces:
            a1T, a3T, a2T = (w1T, w3T, w2T) if ws == 0 else (s1T, s3T, s2T)
            csl = slice(c0, c0 + clen)
            # h = silu(w1 x) * (w3 x), written to hT[:, :, 0:clen]
            for hc in range(HC):
                p1 = h_ps.tile([128, 512], F32, tag="hps")
                for dc in range(DC):
                    nc.tensor.matmul(
                        p1[:, 0:clen], a1T[:, hc, dc], xT[:, dc, csl],
                        start=(dc == 0), stop=(dc == DC - 1),
                    )
                p3 = h_ps.tile([128, 512], F32, tag="hps")
                for dc in range(DC):
                    nc.tensor.matmul(
                        p3[:, 0:clen], a3T[:, hc, dc], xT[:, dc, csl],
                        start=(dc == 0), stop=(dc == DC - 1),
                    )
                sl = silu_p.tile([128, 512], BF16, tag="silu")
                nc.scalar.activation(sl[:, 0:clen], p1[:, 0:clen], AF.Silu)
                nc.vector.tensor_tensor(
                    hT[:, hc, 0:clen], sl[:, 0:clen], p3[:, 0:clen], op=OP.mult
                )

            # y[t, d] = h.T @ w2 for this piece, in 128-token tiles
            for tb in range(clen // 128):
                tsl = slice(tb * 128, (tb + 1) * 128)
                yo = yo_p.tile([128, DIM], F32, tag="yo")
                for dh in range(2):
                    dsl = slice(dh * 384, (dh + 1) * 384)
                    pe = y_ps.tile([128, 384], F32, tag="y")
                    for hc in range(HC):
                        nc.tensor.matmul(
                            pe[:], hT[:, hc, tsl], a2T[:, hc, dsl],
                            start=(hc == 0), stop=(hc == HC - 1),
                        )
                    nc.vector.tensor_copy(yo[:, dsl], pe[:])
                r0 = c0 + tb * 128
                nc.sync.dma_start(y_d[r0:r0 + 128, :], yo[:])

    _split_multi_waits(nc)
    try:
        _CACHE["makespan_ns"] = max(e[2] for e in tc._perfetto_entries)
    except Exception:
        _CACHE["makespan_ns"] = None
    return nc


_CACHE = {}


def _wT_layout(w):
    """[HID, DIM] (bf16) -> DRAM layout [128, HC, DC, 128] where
    [p, hc, dc, i] = w[hc*128 + i, dc*128 + p]."""
    return np.ascontiguousarray(
        w.reshape(HC, 128, DC, 128).transpose(3, 0, 2, 1)
    )


def _w2T_layout(w):
    """[DIM, HID] (bf16) -> DRAM layout [128, HC, DIM] where
    [p, hc, d] = w[d, hc*128 + p]."""
    return np.ascontiguousarray(w.T.reshape(HC, 128, DIM).transpose(1, 0, 2))


def _xT_layout(tok, N):
    """[N, DIM] (bf16) -> DRAM layout [128, DC, N]."""
    return np.ascontiguousarray(tok.T.reshape(DC, 128, N).transpose(1, 0, 2))


def kernel(x, gate_w, w1, w2, w3, ws1, ws2, ws3):
    x = np.asarray(x, dtype=np.float32)
    gate_w = np.asarray(gate_w, dtype=np.float32)
    w1 = np.asarray(w1, dtype=np.float32)
    w2 = np.asarray(w2, dtype=np.float32)
    w3 = np.asarray(w3, dtype=np.float32)
    ws1 = np.asarray(ws1, dtype=np.float32)
    ws2 = np.asarray(ws2, dtype=np.float32)
    ws3 = np.asarray(ws3, dtype=np.float32)

    B, S, D = x.shape
    x2 = np.ascontiguousarray(x.reshape(-1, D))
    Tn = x2.shape[0]
    assert Tn == T and D == DIM

    # --- gate: softmax + top-2 + weight normalization (host)
    logits = x2 @ gate_w.T
    m = logits.max(-1, keepdims=True)
    sm = np.exp(logits - m)
    sm /= sm.sum(-1, keepdims=True)
    ti = np.argsort(-sm, axis=-1)[:, :2]
    tw = np.take_along_axis(sm, ti, axis=-1)
    tw = tw / (tw.sum(-1, keepdims=True) + 1e-20)

    idx_e, cw_e = [], []
    for e in range(E):
        sel = (ti[:, 0] == e) | (ti[:, 1] == e)
        idx = np.nonzero(sel)[0]
        w_tok = np.where(ti[idx, 0] == e, tw[idx, 0], 0.0) + np.where(
            ti[idx, 1] == e, tw[idx, 1], 0.0
        )
        idx_e.append(idx)
        cw_e.append(w_tok.astype(np.float32))

    maxL = max(len(i) for i in idx_e)
    A = max(128, -(-maxL // 128) * 128)
    N = A + B_SH

    key = ("nc", A)
    if key not in _CACHE:
        _CACHE[key] = _build_kernel(A)
    nc = _CACHE[key]
    _CACHE["nc"] = nc

    x_bf = x2.astype(bfloat16)
    sh_w = (
        _wT_layout(ws1.astype(bfloat16)),
        _wT_layout(ws3.astype(bfloat16)),
        _w2T_layout(ws2.astype(bfloat16)),
    )
    in_maps = []
    for c in range(N_CORES):
        idx = idx_e[c]
        tok = np.zeros((N, DIM), dtype=bfloat16)
        tok[: len(idx)] = x_bf[idx]
        tok[A:] = x_bf[c * B_SH:(c + 1) * B_SH]
        in_maps.append({
            "xT": _xT_layout(tok, N),
            "w1T": _wT_layout(w1[c].astype(bfloat16)),
            "w3T": _wT_layout(w3[c].astype(bfloat16)),
            "w2T": _w2T_layout(w2[c].astype(bfloat16)),
            "s1T": sh_w[0],
            "s3T": sh_w[1],
            "s2T": sh_w[2],
        })

    _CACHE["last_in_maps"] = in_maps
    res = run_bass_kernel_spmd(nc, in_maps, list(range(N_CORES)))

    y = np.zeros((T, DIM), dtype=np.float32)
    for c in range(N_CORES):
        yc = np.asarray(res.results[c]["y"], dtype=np.float32)
        idx = idx_e[c]
        y[idx] += cw_e[c][:, None] * yc[: len(idx)]
        y[c * B_SH:(c + 1) * B_SH] += yc[A:]
    return y.reshape(B, S, D)


# revision 6
# speedup vs baseline: 10.3339x; 3.8937x over previous
"""Bass/Trainium2 kernel for nn_MOEFeedForward (8-expert top-2 MoE + shared expert).

Sharding: expert-parallel with host-side dispatch. The host computes the gate
(softmax + top-2) and routes tokens: core c receives expert c's tokens (padded
to capacity A) plus a 1/8 token-slice of the shared-expert work (B=256 tokens).
Every core therefore runs ~A+B token-FFNs of identical shape
(hid=2048, dim=768) — balanced, with no 8x dense overcompute. The host applies
the gate weights and scatter-adds the per-core outputs into the full result.

All operands are pre-transposed/laid out on the host so the device does only
contiguous DMAs and back-to-back bf16 matmuls.

Self-contained: hardcodes shapes from the problem spec.
"""
import sys

sys.path.insert(0, "/opt/trn_rl_repo")

from contextlib import ExitStack

import numpy as np
from ml_dtypes import bfloat16

import concourse.bass as bass
import concourse.tile as tile
from concourse import mybir
from concourse.bass_utils import run_bass_kernel_spmd
from concourse.vector_clock import ScopedClock

DIM = 768
HID = 2048
E = 8
T = 2048
N_CORES = 8
B_SH = T // N_CORES  # shared-expert tokens per core (256)
DC = DIM // 128      # 6 d-chunks
HC = HID // 128      # 16 hid-chunks

F32 = mybir.dt.float32
BF16 = mybir.dt.bfloat16

AF = mybir.ActivationFunctionType
OP = mybir.AluOpType


# ---------------------------------------------------------------------------
# Walrus in this container rejects CTRL instructions (NoOp/Drain) carrying
# more than one sem wait. TileContext's tail drain carries one wait per
# outstanding semaphore. Replace it with a chain of SP nops (one wait each)
# followed by a bare drain.
def _patched_drain_and_barrier(self, tick_clock, wait_clock):
    import bass_rust

    nop_inst = self.nc.sync.nop(nofuse=True, hint="pre_drain_wait_funnel")
    wait_clock.add_sem_waits(
        nop_inst.ins, ScopedClock({None: tick_clock.global_clock})
    )
    si = nop_inst.ins.sync_info
    waits = list(si.on_wait) if si else []
    if len(waits) > 1:
        nop_inst.ins.sync_info.on_wait = waits[:1]
        for w in waits[1:]:
            extra = self.nc.sync.nop(nofuse=True, hint="pre_drain_wait_funnel")
            extra.ins.sync_info = bass_rust.SyncInfo(on_wait=[w], on_update=[])
    self.nc.sync.drain()

    self.nc.all_engine_barrier()
    assert self.sems is not None
    popped = self.nc._tile_sem_poison_stack.pop()
    assert popped is self._sem_poison
    self.nc.clear_and_free_semaphores(list(self.sems.allocated().values()))
    self.nc.all_engine_barrier()


tile.TileContext._drain_and_barrier = _patched_drain_and_barrier


def _split_multi_waits(nc, max_waits=1):
    """This walrus build allows at most one sem wait per instruction. Hoist
    extra waits onto same-engine nops inserted immediately before."""
    import bass_rust

    n_split = 0
    for f in nc.m.functions:
        for bb in f.blocks:
            il = bb.instructions
            i = 0
            while i < len(il):
                inst = il[i]
                si = inst.sync_info
                if si is None or len(si.on_wait) <= max_waits:
                    i += 1
                    continue
                waits = list(si.on_wait)
                si.on_wait = waits[:max_waits]
                for k, w in enumerate(waits[max_waits:]):
                    nop = mybir.InstNoOp(
                        name=f"{inst.name}-wsplit{k}", ins=[], outs=[]
                    )
                    nop.engine = inst.engine
                    nop.sync_info = bass_rust.SyncInfo(on_wait=[w], on_update=[])
                    il.insert(i, nop)
                    i += 1
                n_split += 1
                i += 1
    return n_split
# ---------------------------------------------------------------------------


def _build_kernel(A):
    """A: expert-token capacity (multiple of 64). Columns [0, A) use the
    expert weight set; columns [A, A+B_SH) use the shared weight set.
    Output y is d-major: y_d[p, dc, t] = y[t, dc*128+p]."""
    N = A + B_SH
    nc = bass.Bass()
    xT_d = nc.dram_tensor("xT", [128, DC, N], BF16, kind="ExternalInput")
    w1_d = nc.dram_tensor("w1T", [128, HC, DC, 128], BF16, kind="ExternalInput")
    w3_d = nc.dram_tensor("w3T", [128, HC, DC, 128], BF16, kind="ExternalInput")
    w2_d = nc.dram_tensor("w2T", [128, HC, DC, 128], BF16, kind="ExternalInput")
    s1_d = nc.dram_tensor("s1T", [128, HC, DC, 128], BF16, kind="ExternalInput")
    s3_d = nc.dram_tensor("s3T", [128, HC, DC, 128], BF16, kind="ExternalInput")
    s2_d = nc.dram_tensor("s2T", [128, HC, DC, 128], BF16, kind="ExternalInput")
    y_d = nc.dram_tensor("y", [128, DC, N], F32, kind="ExternalOutput")

    # column pieces: (start, len, weight-set) — expert pieces then shared
    pieces = []
    for c0 in range(0, A, 512):
        pieces.append((c0, min(512, A - c0), 0))
    pieces.append((A, B_SH, 1))

    with tile.TileContext(nc) as tc, ExitStack() as ctx:
        persist = ctx.enter_context(tc.tile_pool(name="persist", bufs=1))
        silu_p = ctx.enter_context(tc.tile_pool(name="silu", bufs=3))
        yo_p = ctx.enter_context(tc.tile_pool(name="yo", bufs=2))
        h_ps = ctx.enter_context(tc.tile_pool(name="h_ps", bufs=4, space="PSUM"))
        y_ps = ctx.enter_context(tc.tile_pool(name="y_ps", bufs=3, space="PSUM"))

        xT = persist.tile([128, DC, N], BF16, tag="xT")
        w1T = persist.tile([128, HC, DC, 128], BF16, tag="w1T")
        w3T = persist.tile([128, HC, DC, 128], BF16, tag="w3T")
        s1T = persist.tile([128, HC, DC, 128], BF16, tag="s1T")
        s3T = persist.tile([128, HC, DC, 128], BF16, tag="s3T")
        w2T = persist.tile([128, HC, DC, 128], BF16, tag="w2T")
        s2T = persist.tile([128, HC, DC, 128], BF16, tag="s2T")
        # hT holds one piece's activations [hid, piece_cols]
        hT = persist.tile([128, HC, 512], BF16, tag="hT")

        # --- DMA schedule: first-piece x, then expert w1/w3 interleaved by
        # hid-chunk (PE consumes chunk-by-chunk), then the rest.
        c00, c0len = pieces[0][0], pieces[0][1]
        nc.sync.dma_start(xT[:, :, c00:c00 + c0len], xT_d[:, :, c00:c00 + c0len])
        for hc in range(HC):
            nc.sync.dma_start(w1T[:, hc], w1_d[:, hc])
            nc.sync.dma_start(w3T[:, hc], w3_d[:, hc])
        if c0len < N:
            nc.sync.dma_start(xT[:, :, c0len:N], xT_d[:, :, c0len:N])
        nc.sync.dma_start(w2T[:], w2_d[:])
        nc.sync.dma_start(s1T[:], s1_d[:])
        nc.sync.dma_start(s3T[:], s3_d[:])
        nc.sync.dma_start(s2T[:], s2_d[:])

        for (c0, clen, ws) in pieces:
            a1T, a3T, a2T = (w1T, w3T, w2T) if ws == 0 else (s1T, s3T, s2T)
            csl = slice(c0, c0 + clen)
            # h = silu(w1 x) * (w3 x), written to hT[:, :, 0:clen]
            for hc in range(HC):
                p1 = h_ps.tile([128, 512], F32, tag="hps")
                for dc in range(DC):
                    nc.tensor.matmul(
                        p1[:, 0:clen], a1T[:, hc, dc], xT[:, dc, csl],
                        start=(dc == 0), stop=(dc == DC - 1),
                    )
                p3 = h_ps.tile([128, 512], F32, tag="hps")
                for dc in range(DC):
                    nc.tensor.matmul(
                        p3[:, 0:clen], a3T[:, hc, dc], xT[:, dc, csl],
                        start=(dc == 0), stop=(dc == DC - 1),
                    )
                sl = silu_p.tile([128, 512], BF16, tag="silu")
                nc.scalar.activation(sl[:, 0:clen], p1[:, 0:clen], AF.Silu)
                nc.vector.tensor_tensor(
                    hT[:, hc, 0:clen], sl[:, 0:clen], p3[:, 0:clen], op=OP.mult
                )

            # y[d, t] = w2.T h for this piece (d-major output)
            for dc in range(DC):
                py = y_ps.tile([128, 512], F32, tag="y")
                for hc in range(HC):
                    nc.tensor.matmul(
                        py[:, 0:clen], a2T[:, hc, dc], hT[:, hc, 0:clen],
                        start=(hc == 0), stop=(hc == HC - 1),
                    )
                yo = yo_p.tile([128, 512], F32, tag="yo")
                nc.vector.tensor_copy(yo[:, 0:clen], py[:, 0:clen])
                nc.sync.dma_start(y_d[:, dc, csl], yo[:, 0:clen])

    _split_multi_waits(nc)
    try:
        _CACHE["makespan_ns"] = max(e[2] for e in tc._perfetto_entries)
    except Exception:
        _CACHE["makespan_ns"] = None
    return nc


_CACHE = {}


def _wT_layout(w):
    """[HID, DIM] (bf16) -> DRAM layout [128, HC, DC, 128] where
    [p, hc, dc, i] = w[hc*128 + i, dc*128 + p]."""
    return np.ascontiguousarray(
        w.reshape(HC, 128, DC, 128).transpose(3, 0, 2, 1)
    )


def _w2T_layout(w):
    """[DIM, HID] (bf16) -> DRAM layout [128, HC, DC, 128] where
    [p, hc, dc, i] = w[dc*128 + i, hc*128 + p]."""
    return np.ascontiguousarray(
        w.T.reshape(HC, 128, DC, 128).transpose(1, 0, 2, 3)
    )


def _xT_layout(tok, N):
    """[N, DIM] (bf16) -> DRAM layout [128, DC, N]."""
    return np.ascontiguousarray(tok.T.reshape(DC, 128, N).transpose(1, 0, 2))


def kernel(x, gate_w, w1, w2, w3, ws1, ws2, ws3):
    x = np.asarray(x, dtype=np.float32)
    gate_w = np.asarray(gate_w, dtype=np.float32)
    w1 = np.asarray(w1, dtype=np.float32)
    w2 = np.asarray(w2, dtype=np.float32)
    w3 = np.asarray(w3, dtype=np.float32)
    ws1 = np.asarray(ws1, dtype=np.float32)
    ws2 = np.asarray(ws2, dtype=np.float32)
    ws3 = np.asarray(ws3, dtype=np.float32)

    B, S, D = x.shape
    x2 = np.ascontiguousarray(x.reshape(-1, D))
    Tn = x2.shape[0]
    assert Tn == T and D == DIM

    # --- gate: softmax + top-2 + weight normalization (host)
    logits = x2 @ gate_w.T
    m = logits.max(-1, keepdims=True)
    sm = np.exp(logits - m)
    sm /= sm.sum(-1, keepdims=True)
    ti = np.argsort(-sm, axis=-1)[:, :2]
    tw = np.take_along_axis(sm, ti, axis=-1)
    tw = tw / (tw.sum(-1, keepdims=True) + 1e-20)

    idx_e, cw_e = [], []
    for e in range(E):
        sel = (ti[:, 0] == e) | (ti[:, 1] == e)
        idx = np.nonzero(sel)[0]
        w_tok = np.where(ti[idx, 0] == e, tw[idx, 0], 0.0) + np.where(
            ti[idx, 1] == e, tw[idx, 1], 0.0
        )
        idx_e.append(idx)
        cw_e.append(w_tok.astype(np.float32))

    maxL = max(len(i) for i in idx_e)
    A = max(128, -(-maxL // 64) * 64)
    N = A + B_SH

    key = ("nc", A)
    if key not in _CACHE:
        _CACHE[key] = _build_kernel(A)
    nc = _CACHE[key]
    _CACHE["nc"] = nc

    x_bf = x2.astype(bfloat16)
    sh_w = (
        _wT_layout(ws1.astype(bfloat16)),
        _wT_layout(ws3.astype(bfloat16)),
        _w2T_layout(ws2.astype(bfloat16)),
    )
    in_maps = []
    for c in range(N_CORES):
        idx = idx_e[c]
        tok = np.zeros((N, DIM), dtype=bfloat16)
        tok[: len(idx)] = x_bf[idx]
        tok[A:] = x_bf[c * B_SH:(c + 1) * B_SH]
        in_maps.append({
            "xT": _xT_layout(tok, N),
            "w1T": _wT_layout(w1[c].astype(bfloat16)),
            "w3T": _wT_layout(w3[c].astype(bfloat16)),
            "w2T": _w2T_layout(w2[c].astype(bfloat16)),
            "s1T": sh_w[0],
            "s3T": sh_w[1],
            "s2T": sh_w[2],
        })

    _CACHE["last_in_maps"] = in_maps
    res = run_bass_kernel_spmd(nc, in_maps, list(range(N_CORES)))

    y = np.zeros((T, DIM), dtype=np.float32)
    for c in range(N_CORES):
        yc_dm = np.asarray(res.results[c]["y"], dtype=np.float32)  # [128, DC, N]
        yc = yc_dm.transpose(1, 0, 2).reshape(DIM, N).T  # [N, DIM]
        idx = idx_e[c]
        y[idx] += cw_e[c][:, None] * yc[: len(idx)]
        y[c * B_SH:(c + 1) * B_SH] += yc[A:]
    return y.reshape(B, S, D)
